# Initial kernel scaffold
#
"""BatchHardTripletLoss on 8 Trainium2 NeuronCores.

Strategy (data-parallel over anchor rows):
  - core c owns anchor rows [c*512, (c+1)*512) of the 4096x4096 distance matrix
  - each core receives the full embedding matrix transposed (K-major, fp16,
    scaled by sqrt(2)) plus its own 512-column stationary block, and a per-core
    f32 "mask" tile holding every additive term of d2 (|x|^2 row/col terms, the
    eps terms) plus a +PEN penalty on same-label pairs.
  - on device:  w[m,n] = mask[m,n] - (sqrt2*E_m) . (sqrt2*E_n)  = d2 + PEN*same
    row-max of w  -> PEN + d2 of hardest positive   (penalty dominates)
    row-min of w  -> d2 of hardest negative         (same-label rows lifted out)
    both reductions are fused into the elementwise pass via tensor_tensor_reduce.
  - host: subtract PEN, sqrt, validity via label bincount, masked mean.
"""

import numpy as np

import concourse.bass as bass
import concourse.mybir as mybir
from concourse.bass_utils import run_bass_kernel_spmd
from concourse.tile import TileContext

B = 4096          # batch (anchors)
D = 512           # embedding dim
N_CORES = 8
ROWS = B // N_CORES      # 512 anchor rows per core
P = 128                  # partitions
MT = ROWS // P           # 4 m-tiles per core
NW = 512                 # n-tile width (one PSUM bank of fp32)
NT = B // NW             # 8 n-tiles
KT = D // P              # 4 contraction tiles

PEN = 16384.0            # same-label penalty; must exceed max d2 (~2.7k here)
MARGIN = 0.5
EPS = 1e-6
NEG_INIT = -3.0e38
POS_INIT = 3.0e38

_nc_cache = []


def _build():
    nc = bass.Bass("TRN2", target_bir_lowering=False)
    fp16 = mybir.dt.float16
    f32 = mybir.dt.float32

    et = nc.dram_tensor("et", [D, B], fp16, kind="ExternalInput")
    eblk = nc.dram_tensor("eblk", [D, ROWS], fp16, kind="ExternalInput")
    maskd = nc.dram_tensor("maskd", [ROWS, B], f32, kind="ExternalInput")
    outd = nc.dram_tensor("out", [2, MT, P], f32, kind="ExternalOutput")

    with TileContext(nc) as tc:
        with (
            tc.tile_pool(name="etp", bufs=KT) as etp,
            tc.tile_pool(name="ebp", bufs=KT) as ebp,
            tc.tile_pool(name="maskp", bufs=2) as maskp,
            tc.tile_pool(name="wp", bufs=2) as wp,
            tc.tile_pool(name="accp", bufs=2) as accp,
            tc.tile_pool(name="psp", bufs=4, space="PSUM") as psp,
        ):
            et_sb, eb_sb = [], []
            for k in range(KT):
                tk = etp.tile([P, B], fp16, tag=f"et{k}")
                nc.sync.dma_start(out=tk, in_=et[k * P:(k + 1) * P, :])
                et_sb.append(tk)
                bk = ebp.tile([P, ROWS], fp16, tag=f"eb{k}")
                nc.sync.dma_start(out=bk, in_=eblk[k * P:(k + 1) * P, :])
                eb_sb.append(bk)

            for t in range(MT):
                mask_sb = maskp.tile([P, B], f32)
                nc.sync.dma_start(out=mask_sb, in_=maskd[t * P:(t + 1) * P, :])
                w_sb = wp.tile([P, B], f32)
                hp_acc = accp.tile([P, NT], f32, tag="hp")
                hn_acc = accp.tile([P, 1], f32, tag="hn")

                for j in range(NT):
                    ps = psp.tile([P, NW], f32)
                    for k in range(KT):
                        nc.tensor.matmul(
                            ps,
                            eb_sb[k][:, t * P:(t + 1) * P],
                            et_sb[k][:, j * NW:(j + 1) * NW],
                            start=(k == 0),
                            stop=(k == KT - 1),
                        )
                    # w = mask - gram2 ; running row-max -> hardest positive
                    nc.vector.tensor_tensor_reduce(
                        out=w_sb[:, j * NW:(j + 1) * NW],
                        in0=mask_sb[:, j * NW:(j + 1) * NW],
                        in1=ps,
                        scale=1.0,
                        scalar=(NEG_INIT if j == 0 else hp_acc[:, j - 1:j]),
                        op0=mybir.AluOpType.subtract,
                        op1=mybir.AluOpType.max,
                        accum_out=hp_acc[:, j:j + 1],
                    )
                # one wide pass: row-min of w -> hardest negative
                nc.vector.tensor_tensor_reduce(
                    out=w_sb,
                    in0=w_sb,
                    in1=w_sb,
                    scale=1.0,
                    scalar=POS_INIT,
                    op0=mybir.AluOpType.min,
                    op1=mybir.AluOpType.min,
                    accum_out=hn_acc[:, 0:1],
                )
                nc.sync.dma_start(out=outd[0, t, :], in_=hp_acc[:, NT - 1:NT])
                nc.sync.dma_start(out=outd[1, t, :], in_=hn_acc[:, 0:1])
    return nc


def _get_nc():
    if not _nc_cache:
        _nc_cache.append(_build())
    return _nc_cache[0]


def _prepare_inputs(embeddings, labels):
    Ef = np.ascontiguousarray(np.asarray(embeddings, dtype=np.float32))
    lab = np.asarray(labels).astype(np.int64)
    sq = np.sum(Ef * Ef, axis=1, dtype=np.float32)          # [B]
    s = np.sum(Ef, axis=1, dtype=np.float32)                # [B]
    et16 = np.ascontiguousarray((Ef * np.float32(np.sqrt(2.0))).T.astype(np.float16))

    col_term = (sq - 2.0 * EPS * s).astype(np.float32)      # column-dependent
    row_term = (sq + 2.0 * EPS * s + D * EPS * EPS).astype(np.float32)

    in_maps = []
    for c in range(N_CORES):
        r0, r1 = c * ROWS, (c + 1) * ROWS
        mask = row_term[r0:r1, None] + col_term[None, :]
        mask = mask + np.float32(PEN) * (lab[r0:r1, None] == lab[None, :])
        in_maps.append({
            "et": et16,
            "eblk": np.ascontiguousarray(et16[:, r0:r1]),
            "maskd": np.ascontiguousarray(mask.astype(np.float32)),
        })
    return in_maps, lab


def _postprocess(results, lab):
    hp_raw = np.concatenate([r["out"][0].reshape(-1) for r in results])  # [B]
    hn_raw = np.concatenate([r["out"][1].reshape(-1) for r in results])  # [B]
    hp = np.sqrt(np.maximum(hp_raw - np.float32(PEN), 0.0, dtype=np.float32))
    hn = np.sqrt(np.maximum(hn_raw, 0.0, dtype=np.float32))

    cnt_lab = np.bincount(lab, minlength=1)
    n_same = cnt_lab[lab]
    valid = (n_same > 1) & (n_same < B)
    per = np.where(valid, np.maximum(hp - hn + np.float32(MARGIN), 0.0), 0.0)
    cnt = np.float32(valid.sum())
    if cnt > 0:
        loss = np.float32(per.sum(dtype=np.float32) / max(cnt, np.float32(1.0)))
    else:
        loss = np.float32(0.0)
    return np.asarray(loss, dtype=np.float32)


def _run(in_maps, **kw):
    nc = _get_nc()
    return run_bass_kernel_spmd(nc, in_maps, core_ids=list(range(N_CORES)), **kw)


def kernel(embeddings, labels):
    in_maps, lab = _prepare_inputs(embeddings, labels)
    res = _run(in_maps)
    return _postprocess(res.results, lab)


# revision 14
# speedup vs baseline: 12.6981x; 12.6981x over previous
"""BatchHardTripletLoss on 8 Trainium2 NeuronCores.

Strategy (data-parallel over anchor rows):
  - core c owns anchor rows [c*512, (c+1)*512) of the 4096x4096 distance matrix
  - each core receives the full embedding matrix transposed (K-major, fp16,
    scaled by sqrt(2)) plus its own 512-column stationary block, and a per-core
    f32 "mask" tile holding every additive term of d2 (|x|^2 row/col terms, the
    eps terms) plus a +PEN penalty on same-label pairs.
  - on device:  w[m,n] = mask[m,n] - (sqrt2*E_m) . (sqrt2*E_n)  = d2 + PEN*same
    row-max of w  -> PEN + d2 of hardest positive   (penalty dominates)
    row-min of w  -> d2 of hardest negative         (same-label rows lifted out)
    both reductions are fused into the elementwise pass via tensor_tensor_reduce.
  - host: subtract PEN, sqrt, validity via label bincount, masked mean.
"""

import numpy as np

import concourse.bacc as bacc
import concourse.mybir as mybir
from concourse.bass_utils import run_bass_kernel_spmd
from concourse.tile import TileContext

B = 4096          # batch (anchors)
D = 512           # embedding dim
N_CORES = 8
ROWS = B // N_CORES      # 512 anchor rows per core
P = 128                  # partitions
MT = ROWS // P           # 4 m-tiles per core
NW = 512                 # n-tile width (one PSUM bank of fp32)
NT = B // NW             # 8 n-tiles
KT = D // P              # 4 contraction tiles

PEN = 16384.0            # same-label penalty; must exceed max d2 (~2.7k here)
MARGIN = 0.5
EPS = 1e-6
NEG_INIT = -3.0e38
POS_INIT = 3.0e38

_nc_cache = []


def _build(reps=1):
    nc = bacc.Bacc("TRN2", target_bir_lowering=False)
    fp16 = mybir.dt.float16
    f32 = mybir.dt.float32

    et = nc.dram_tensor("et", [D, B], fp16, kind="ExternalInput")
    eblk = nc.dram_tensor("eblk", [D, ROWS], fp16, kind="ExternalInput")
    maskd = nc.dram_tensor("maskd", [ROWS, B], f32, kind="ExternalInput")
    outd = nc.dram_tensor("out", [2, MT, P], f32, kind="ExternalOutput")

    with TileContext(nc) as tc:
        with (
            tc.tile_pool(name="etp", bufs=1) as etp,
            tc.tile_pool(name="ebp", bufs=1) as ebp,
            tc.tile_pool(name="maskp", bufs=MT) as maskp,
            tc.tile_pool(name="wp", bufs=2) as wp,
            tc.tile_pool(name="accp", bufs=MT) as accp,
            tc.tile_pool(name="primep", bufs=MT) as primep,
            tc.tile_pool(name="psp", bufs=4, space="PSUM") as psp,
        ):
            et_sb, eb_sb = [], []
            for k in range(KT):
                tk = etp.tile([P, B], fp16, tag=f"et{k}")
                nc.sync.dma_start(out=tk, in_=et[k * P:(k + 1) * P, :])
                et_sb.append(tk)
                bk = ebp.tile([P, ROWS], fp16, tag=f"eb{k}")
                nc.sync.dma_start(out=bk, in_=eblk[k * P:(k + 1) * P, :])
                eb_sb.append(bk)

            for t in [t for _ in range(reps) for t in range(MT)]:
                mask_sb = maskp.tile([P, B], f32)
                nc.gpsimd.dma_start(out=mask_sb, in_=maskd[t * P:(t + 1) * P, :])
                # absorb the DMA wait on DVE so later DVE ops only wait on PE
                prime = primep.tile([P, 1], f32, tag="prime")
                nc.vector.tensor_copy(prime, mask_sb[:, 0:1])
                w_sb = wp.tile([P, B], f32)
                hp_acc = accp.tile([P, 1], f32, tag="hp")
                hn_acc = accp.tile([P, 1], f32, tag="hn")

                for j in range(NT):
                    ps = psp.tile([P, NW], f32)
                    for k in range(KT):
                        nc.tensor.matmul(
                            ps,
                            eb_sb[k][:, t * P:(t + 1) * P],
                            et_sb[k][:, j * NW:(j + 1) * NW],
                            start=(k == 0),
                            stop=(k == KT - 1),
                        )
                    # w = mask - gram2  (= d2 + PEN*same)
                    nc.vector.tensor_sub(
                        w_sb[:, j * NW:(j + 1) * NW],
                        mask_sb[:, j * NW:(j + 1) * NW],
                        ps,
                    )
                # row-max of w -> hardest positive (+PEN); row-min -> hardest neg
                nc.vector.tensor_reduce(
                    hp_acc[:, 0:1], w_sb, mybir.AxisListType.X, mybir.AluOpType.max
                )
                nc.vector.tensor_reduce(
                    hn_acc[:, 0:1], w_sb, mybir.AxisListType.X, mybir.AluOpType.min
                )
                nc.sync.dma_start(out=outd[0, t, :], in_=hp_acc[:, 0:1])
                nc.sync.dma_start(out=outd[1, t, :], in_=hn_acc[:, 0:1])
    nc.compile()
    return nc


def _get_nc():
    if not _nc_cache:
        _nc_cache.append(_build())
    return _nc_cache[0]


def _prepare_inputs(embeddings, labels):
    Ef = np.ascontiguousarray(np.asarray(embeddings, dtype=np.float32))
    lab = np.asarray(labels).astype(np.int64)
    sq = np.sum(Ef * Ef, axis=1, dtype=np.float32)          # [B]
    s = np.sum(Ef, axis=1, dtype=np.float32)                # [B]
    et16 = np.ascontiguousarray((Ef * np.float32(np.sqrt(2.0))).T.astype(np.float16))

    col_term = (sq - 2.0 * EPS * s).astype(np.float32)      # column-dependent
    row_term = (sq + 2.0 * EPS * s + D * EPS * EPS).astype(np.float32)

    in_maps = []
    for c in range(N_CORES):
        r0, r1 = c * ROWS, (c + 1) * ROWS
        mask = row_term[r0:r1, None] + col_term[None, :]
        mask = mask + np.float32(PEN) * (lab[r0:r1, None] == lab[None, :])
        in_maps.append({
            "et": et16,
            "eblk": np.ascontiguousarray(et16[:, r0:r1]),
            "maskd": np.ascontiguousarray(mask.astype(np.float32)),
        })
    return in_maps, lab


def _postprocess(results, lab):
    hp_raw = np.concatenate([r["out"][0].reshape(-1) for r in results])  # [B]
    hn_raw = np.concatenate([r["out"][1].reshape(-1) for r in results])  # [B]
    hp = np.sqrt(np.maximum(hp_raw - np.float32(PEN), 0.0, dtype=np.float32))
    hn = np.sqrt(np.maximum(hn_raw, 0.0, dtype=np.float32))

    cnt_lab = np.bincount(lab, minlength=1)
    n_same = cnt_lab[lab]
    valid = (n_same > 1) & (n_same < B)
    per = np.where(valid, np.maximum(hp - hn + np.float32(MARGIN), 0.0), 0.0)
    cnt = np.float32(valid.sum())
    if cnt > 0:
        loss = np.float32(per.sum(dtype=np.float32) / max(cnt, np.float32(1.0)))
    else:
        loss = np.float32(0.0)
    return np.asarray(loss, dtype=np.float32)


def _run(in_maps, **kw):
    nc = _get_nc()
    return run_bass_kernel_spmd(nc, in_maps, core_ids=list(range(N_CORES)), **kw)


def kernel(embeddings, labels):
    in_maps, lab = _prepare_inputs(embeddings, labels)
    res = _run(in_maps)
    return _postprocess(res.results, lab)
